# revision 1
# baseline (speedup 1.0000x reference)
"""Trainium2 Bass kernel for nn_CNNBackbone: conv1d(D->C,K=5) + BN + ReLU,
conv1d(C->C,K=5) + BN + ReLU, conv1d(C->D,1x1), masked mean over ragged lengths.

Strategy
--------
Data-parallel over batch: 32 samples -> 8 cores x 4 sample-slots.
Samples are sorted by (masked) length and assigned snake-style so each slot's
group of 8 (one per core) has near-uniform length; per-slot loop bounds are
compile-time constants derived from the group max (SPMD: one program, 8 cores).

Algebraic simplifications (host side):
 - BN folded into conv weights/biases.
 - The final 1x1 conv commutes with the masked sum:
     pooled = Wf @ (sum_{t<len} h2[:, t]) / len + bf        (len > 0)
   so h2 is reduced on-chip (fused into the conv2 ReLU epilogue via
   ScalarE accum_out; partially-masked chunks use a 0/1 mask multiply +
   reduce on DVE), and the 1x1 conv becomes a tiny per-sample fp32 matvec.
 - Computation is truncated past each slot-group's max length (rounded to 16).

Conv-as-matmul: channels on partitions, 5 taps = 5 PSUM-accumulated matmuls
with shifted rhs slices; bf16 operands, fp32 PSUM.
"""

import math

import numpy as np
import ml_dtypes

import concourse.bass as bass
import concourse.mybir as mybir
import concourse.tile as tile
from concourse import bacc
from concourse.bass_utils import run_bass_kernel_spmd

B, S, D, C, KW = 32, 2048, 128, 256, 5
P = 128
CH = 512            # full chunk (matmul free dim / PSUM bank)
GR = 16             # tail-chunk width granularity
NCORES = 8
NSLOTS = B // NCORES
CB = C // P         # channel blocks of 128
EPS = 1e-5
H0W = S + 4         # x^T buffer width (2 halo cols each side)
H1W = S + 8         # h1 buffer width
BF16 = ml_dtypes.bfloat16
F32 = mybir.dt.float32
BF = mybir.dt.bfloat16

_BUILD_CACHE: dict = {}
LAST_RESULTS = None  # BassKernelResults of the most recent run (for test harness)
TRACE = False        # set True (or env BASS_TRACE=1) to capture a profile


def _chunks(total):
    """Split `total` columns into 512-wide chunks plus a short tail."""
    ws = [CH] * (total // CH)
    if total % CH:
        ws.append(total % CH)
    return ws


def _build(slot_cfg):
    """Build + compile the SPMD Bass program.

    slot_cfg[j] = (L1, L2, c0): conv1/conv2 computed column counts (multiples
    of GR) and the count of fully-unmasked 512-chunks for slot j's group.
    """
    nc = bacc.Bacc(None, target_bir_lowering=False, debug=False)

    xT = nc.dram_tensor("xT", [NSLOTS, P, S], BF, kind="ExternalInput")
    msk = nc.dram_tensor("msk", [NSLOTS, S], BF, kind="ExternalInput")
    w1t = nc.dram_tensor("w1t", [P, KW, CB, P], BF, kind="ExternalInput")
    w2t = nc.dram_tensor("w2t", [P, KW, CB, CB, P], BF, kind="ExternalInput")
    wft = nc.dram_tensor("wft", [P, CB, P], F32, kind="ExternalInput")
    bias1 = nc.dram_tensor("bias1", [P, CB], F32, kind="ExternalInput")
    bias2 = nc.dram_tensor("bias2", [P, CB], F32, kind="ExternalInput")
    invl = nc.dram_tensor("invl", [P, NSLOTS], F32, kind="ExternalInput")
    bfe = nc.dram_tensor("bfe", [P, NSLOTS], F32, kind="ExternalInput")
    out = nc.dram_tensor("out", [P, NSLOTS], F32, kind="ExternalOutput")

    RELU = mybir.ActivationFunctionType.Relu
    ADD = mybir.AluOpType.add
    MUL = mybir.AluOpType.mult

    with tile.TileContext(nc) as tc:
        with (
            tc.tile_pool(name="consts", bufs=1) as consts,
            tc.tile_pool(name="h0p", bufs=2) as h0p,
            tc.tile_pool(name="h1p", bufs=2) as h1p,
            tc.tile_pool(name="mkp", bufs=2) as mkp,
            tc.tile_pool(name="scp", bufs=4) as scp,
            tc.tile_pool(name="psp", bufs=7, space="PSUM") as psp,
            tc.tile_pool(name="psv", bufs=1, space="PSUM") as psv,
        ):
            w1s = consts.tile([P, KW, CB, P], BF)
            w2s = consts.tile([P, KW, CB, CB, P], BF)
            wfs = consts.tile([P, CB, P], F32)
            b1s = consts.tile([P, CB], F32)
            b2s = consts.tile([P, CB], F32)
            invls = consts.tile([P, NSLOTS], F32)
            bfes = consts.tile([P, NSLOTS], F32)
            rowsums = consts.tile([P, NSLOTS, CB, S // CH + 1], F32)
            rs_red = consts.tile([P, NSLOTS, CB], F32)
            out_sb = consts.tile([P, NSLOTS], F32)

            h0_t = [None] * NSLOTS
            h1_t = [None] * NSLOTS
            mk_t = [None] * NSLOTS

            def emit_load(j, split_first=False):
                L1, L2, c0 = slot_cfg[j]
                if L1 == 0:
                    return
                h0 = h0p.tile([P, H0W], BF, tag="h0")
                h1 = h1p.tile([P, CB, H1W], BF, tag="h1")
                h0_t[j], h1_t[j] = h0, h1
                w = min(L1 + 2, S)
                if split_first:
                    # first chunk (+halo) on the scalar queue, remaining
                    # chunks as separate pieces on sync — each matmul chunk
                    # only waits for its own piece.
                    w0 = min(CH + 6, w)
                    nc.scalar.dma_start(h0[:, 2 : 2 + w0], xT[j, :, 0:w0])
                    a = w0
                    while a < w:
                        b_ = min(a + CH, w)
                        nc.sync.dma_start(h0[:, 2 + a : 2 + b_], xT[j, :, a:b_])
                        a = b_
                else:
                    nc.sync.dma_start(h0[:, 2 : 2 + w], xT[j, :, 0:w])
                nc.vector.memset(h0[:, 0:2], 0.0)
                if 2 + w < L1 + 4:
                    nc.vector.memset(h0[:, 2 + w : L1 + 4], 0.0)
                for cb in range(CB):
                    nc.vector.memset(h1[:, cb, 0:2], 0.0)
                    nc.vector.memset(h1[:, cb, 2 + L1 : 4 + L1], 0.0)
            def emit_mask(j):
                L1, L2, c0 = slot_cfg[j]
                wm = L2 - c0 * CH
                if L1 > 0 and wm > 0:
                    mk = mkp.tile([P, S], BF, tag="mk")
                    mk_t[j] = mk
                    src = msk[j, c0 * CH : c0 * CH + wm]
                    bsrc = bass.AP(
                        tensor=src.tensor, offset=src.offset,
                        ap=[[0, P]] + list(src.ap),
                    )
                    nc.gpsimd.dma_start(mk[:, 0:wm], bsrc)

            def emit_conv1(j):
                L1, L2, c0 = slot_cfg[j]
                h0, h1 = h0_t[j], h1_t[j]
                for c, wc in enumerate(_chunks(L1)):
                    for cb in range(CB):
                        ps = psp.tile([P, CH], F32, tag="ps")
                        for k in range(KW):
                            nc.tensor.matmul(
                                ps[:, 0:wc],
                                w1s[:, k, cb, :],
                                h0[:, c * CH + k : c * CH + k + wc],
                                start=(k == 0),
                                stop=(k == KW - 1),
                            )
                        nc.scalar.activation(
                            h1[:, cb, 2 + c * CH : 2 + c * CH + wc],
                            ps[:, 0:wc], RELU, bias=b1s[:, cb : cb + 1],
                        )

            def emit_conv2(j, full_last=False):
                L1, L2, c0 = slot_cfg[j]
                h1, mk = h1_t[j], mk_t[j]
                order = list(enumerate(_chunks(L2)))
                if full_last:
                    # masked chunks (long DVE epilogue chains) first, fully
                    # accumulated chunks (single fused ACT) last, so the
                    # kernel tail only waits on the short chain.
                    order = [cw for cw in order if cw[0] >= c0] + \
                            [cw for cw in order if cw[0] < c0]
                for c, wc in order:
                    for cb in range(CB):
                        ps = psp.tile([P, CH], F32, tag="ps")
                        idx = 0
                        for cib in range(CB):
                            for k in range(KW):
                                nc.tensor.matmul(
                                    ps[:, 0:wc],
                                    w2s[:, k, cib, cb, :],
                                    h1[:, cib, c * CH + k : c * CH + k + wc],
                                    start=(idx == 0),
                                    stop=(idx == CB * KW - 1),
                                )
                                idx += 1
                        col = rowsums[:, j, cb, c : c + 1]
                        h2 = scp.tile([P, CH], BF, tag="h2")
                        if c < c0:
                            # unmasked for every core in the group:
                            # ReLU + bias + rowsum fused on ScalarE
                            nc.scalar.activation(
                                h2[:, 0:wc], ps[:, 0:wc], RELU,
                                bias=b2s[:, cb : cb + 1], accum_out=col,
                            )
                        else:
                            nc.scalar.activation(
                                h2[:, 0:wc], ps[:, 0:wc], RELU,
                                bias=b2s[:, cb : cb + 1],
                            )
                            sc = scp.tile([P, CH], BF, tag="sc")
                            mslice = mk[:, (c - c0) * CH : (c - c0) * CH + wc]
                            nc.vector.tensor_tensor(
                                sc[:, 0:wc], h2[:, 0:wc], mslice, MUL,
                            )
                            nc.vector.tensor_reduce(
                                col, sc[:, 0:wc],
                                axis=mybir.AxisListType.X, op=ADD,
                            )

            def emit_slot_reduce(j):
                L1, L2, c0 = slot_cfg[j]
                n2c = len(_chunks(L2))
                for cb in range(CB):
                    if n2c == 0:
                        nc.vector.memset(rs_red[:, j, cb : cb + 1], 0.0)
                    elif n2c == 1:
                        nc.vector.tensor_copy(
                            rs_red[:, j, cb : cb + 1], rowsums[:, j, cb, 0:1]
                        )
                    else:
                        nc.vector.tensor_reduce(
                            rs_red[:, j, cb : cb + 1],
                            rowsums[:, j, cb, 0:n2c],
                            axis=mybir.AxisListType.X, op=ADD,
                        )
                # fold 1/len here so the kernel tail only does matvec + bias
                nc.vector.tensor_tensor(
                    rs_red[:, j, :], rs_red[:, j, :],
                    invls[:, j : j + 1].to_broadcast((P, CB)), MUL,
                )

            # ---- emission order ----
            # PE warmup: the first data DMAs cannot complete before ~4us of
            # per-partition descriptor processing, so spend that dead window
            # on dummy matmuls. 9 x 512 cols at the cold rate is ~3.8us of
            # sustained PE activity -- enough to flip the HAM clock gate to
            # 8/8 (2.4 GHz) before the first real matmul issues.
            warm_w = scp.tile([P, CH], BF, tag="warm")
            warm_ps = psp.tile([P, CH], F32, tag="ps")
            nc.gpsimd.memset(warm_w, 0.0)
            for _ in range(9):
                nc.tensor.matmul(warm_ps, warm_w[:, 0:P], warm_w,
                                 start=True, stop=True)

            # cb0 taps on the sync HWDGE queue (ahead of x pieces), cb1 taps
            # on gpsimd SWDGE; slot 0's first x chunk goes on the scalar
            # queue (behind only the ACT table load). The first 5 matmuls
            # (cb0, chunk 0) then have the earliest possible start.
            nc.sync.dma_start(w1s[:, :, 0, :], w1t[:, :, 0, :])
            nc.gpsimd.dma_start(w1s[:, :, 1, :], w1t[:, :, 1, :])
            emit_load(0, split_first=True)
            nc.gpsimd.dma_start(w2s, w2t[:])
            nc.scalar.dma_start(b1s, bias1[:])
            nc.scalar.dma_start(b2s, bias2[:])
            nc.scalar.dma_start(invls, invl[:])
            emit_load(1)
            emit_conv1(0)
            emit_mask(0)
            emit_mask(1)
            emit_load(2)
            emit_conv1(1)
            emit_conv2(0)
            emit_slot_reduce(0)
            emit_load(3)
            emit_mask(2)
            emit_mask(3)
            emit_conv1(2)
            emit_conv2(1)
            emit_slot_reduce(1)
            nc.scalar.dma_start(wfs, wft[:])
            nc.scalar.dma_start(bfes, bfe[:])
            # finals: 1x1-conv matvec (fp32) per sample; slots 0/1 are issued
            # before the last conv blocks so only the final slots' chain sits
            # on the kernel tail.
            pooled = psv.tile([P, NSLOTS], F32)

            def emit_matvec(j):
                for cb in range(CB):
                    nc.tensor.matmul(
                        pooled[:, j : j + 1],
                        wfs[:, cb, :],
                        rs_red[:, j, cb : cb + 1],
                        start=(cb == 0),
                        stop=(cb == CB - 1),
                    )

            emit_conv1(3)
            emit_conv2(3)
            emit_slot_reduce(3)
            emit_matvec(0)
            emit_matvec(1)
            emit_conv2(2, full_last=True)
            emit_slot_reduce(2)
            emit_matvec(3)
            emit_matvec(2)
            nc.vector.tensor_tensor(out_sb, pooled, bfes, ADD)
            nc.sync.dma_start(out[:], out_sb)

    nc.compile()
    return nc


def _prep(inputs):
    """Host-side: BN folding, weight packing, length-sorted slot assignment."""
    x = np.ascontiguousarray(np.asarray(inputs["x"], dtype=np.float32))
    spi = np.asarray(inputs["start_padding_indices"]).astype(np.int64).reshape(B)
    W1 = np.asarray(inputs["W1"], np.float32)
    b1 = np.asarray(inputs["b1"], np.float32)
    g1 = np.asarray(inputs["g1"], np.float32)
    be1 = np.asarray(inputs["be1"], np.float32)
    m1 = np.asarray(inputs["m1"], np.float32)
    v1 = np.asarray(inputs["v1"], np.float32)
    W2 = np.asarray(inputs["W2"], np.float32)
    b2 = np.asarray(inputs["b2"], np.float32)
    g2 = np.asarray(inputs["g2"], np.float32)
    be2 = np.asarray(inputs["be2"], np.float32)
    m2 = np.asarray(inputs["m2"], np.float32)
    v2 = np.asarray(inputs["v2"], np.float32)
    Wf = np.asarray(inputs["Wf"], np.float32)
    bf = np.asarray(inputs["bf"], np.float32)

    lens = np.where(spi == -1, S, spi)
    lens = np.clip(lens, 0, S).astype(np.int64)

    order = np.argsort(-lens, kind="stable")
    assign = order.reshape(NSLOTS, NCORES)  # [slot, core] -> sample idx

    slot_cfg = []
    for j in range(NSLOTS):
        lj = lens[assign[j]]
        lmax, lmin = int(lj.max()), int(lj.min())
        if lmax == 0:
            slot_cfg.append((0, 0, 0))
            continue
        L2 = min(math.ceil(lmax / GR) * GR, S)
        L1 = min(math.ceil(min(lmax + 2, S) / GR) * GR, S)
        c0 = min(lmin // CH, len(_chunks(L2)))
        slot_cfg.append((L1, L2, c0))
    slot_cfg = tuple(slot_cfg)

    # fold BN into conv weights/biases
    s1 = g1 / np.sqrt(v1 + EPS)
    W1f = W1 * s1[:, None, None]
    b1f = (b1 - m1) * s1 + be1
    s2 = g2 / np.sqrt(v2 + EPS)
    W2f = W2 * s2[:, None, None]
    b2f = (b2 - m2) * s2 + be2

    # pack weights: lhsT layouts (contraction channel on partitions)
    w1t = np.ascontiguousarray(
        W1f.reshape(CB, P, D, KW).transpose(2, 3, 0, 1)
    ).astype(BF16)  # [d, k, cb, co]
    w2t = np.ascontiguousarray(
        W2f.reshape(CB, P, CB, P, KW).transpose(3, 4, 2, 0, 1)
    ).astype(BF16)  # [ci, k, cib, cob, co]
    wft = np.ascontiguousarray(
        Wf[:, :, 0].reshape(D, CB, P).transpose(2, 1, 0)
    ).astype(np.float32)  # [ci, cib, d]
    bias1 = np.ascontiguousarray(b1f.reshape(CB, P).T).astype(np.float32)
    bias2 = np.ascontiguousarray(b2f.reshape(CB, P).T).astype(np.float32)

    t_idx = np.arange(S)
    in_maps = []
    for i in range(NCORES):
        xT_i = np.empty((NSLOTS, P, S), dtype=BF16)
        msk_i = np.zeros((NSLOTS, S), dtype=BF16)
        invl_i = np.empty((P, NSLOTS), dtype=np.float32)
        bfe_i = np.empty((P, NSLOTS), dtype=np.float32)
        for j in range(NSLOTS):
            b_idx = int(assign[j, i])
            L = int(lens[b_idx])
            xT_i[j] = x[b_idx].T.astype(BF16)
            msk_i[j] = (t_idx < L).astype(BF16)
            invl_i[:, j] = 1.0 / max(L, 1)
            bfe_i[:, j] = bf * (1.0 if L > 0 else 0.0)
        in_maps.append({
            "xT": xT_i, "msk": msk_i,
            "w1t": w1t, "w2t": w2t, "wft": wft,
            "bias1": bias1, "bias2": bias2,
            "invl": invl_i, "bfe": bfe_i,
        })
    return slot_cfg, assign, in_maps


def kernel(**inputs) -> np.ndarray:
    global LAST_RESULTS
    import os

    slot_cfg, assign, in_maps = _prep(inputs)
    nc = _BUILD_CACHE.get(slot_cfg)
    if nc is None:
        nc = _build(slot_cfg)
        _BUILD_CACHE[slot_cfg] = nc

    trace = TRACE or bool(os.environ.get("BASS_TRACE"))
    if trace:
        try:
            import antenv.axon_hooks  # noqa: F401  (absent in some containers)
        except ImportError:
            trace = False
    res = run_bass_kernel_spmd(
        nc, in_maps, core_ids=list(range(NCORES)), trace=trace,
    )
    LAST_RESULTS = res

    pooled = np.zeros((B, D), dtype=np.float32)
    for i in range(NCORES):
        out_i = np.asarray(res.results[i]["out"], dtype=np.float32)  # [P, NSLOTS]
        for j in range(NSLOTS):
            pooled[int(assign[j, i])] = out_i[:, j]
    return pooled

